# revision 22
# baseline (speedup 1.0000x reference)
"""ChannelDropout Bass kernel for 8 Trainium2 NeuronCores.

out[b,c,t] = x[b,c,t] * scale[b,c] with
  scale = valid * (dist0 > 0.1) / (mean_{n=1..100}((dist_n > 0.1)|~valid) + 1e-8)

Data-parallel over the batch dim: 8 batches per core; the per-core x shard
is a contiguous [2200, 3000] f32 block. Phase 1 computes the per-row scale
from positions/centers (tiny); phase 2 streams x through SBUF with a
per-partition scalar multiply (memory-bound, the roofline).

Written in raw Bass (not Tile): this toolchain's walrus build allows at
most ONE semaphore wait per instruction, which Tile's scheduler does not
respect. Also, consecutive instructions on the same engine have no RAW
interlock, so dependent compute ops are chained through a progress
semaphore. Pipeline: sync engine issues x loads (HWDGE/SP ring), vector
engine does all compute, scalar engine issues stores (HWDGE/ACT ring),
NBUF-deep buffered SBUF x tiles with per-slot DMA sems.
"""

from contextlib import ExitStack

import numpy as np

import concourse.bass as bass
from concourse import mybir
from concourse.bass_utils import run_bass_kernel_spmd

B, C, T, D = 64, 275, 3000, 2
NCEN = 101                 # centers rows (0 = ban center, 1..100 Monte-Carlo)
N_CORES = 8
B_SH = B // N_CORES        # 8 batches per core
ROWS = B_SH * C            # 2200 rows per core
P = 128
FULL = ROWS // P           # 17 full 128-row tiles
TAIL = ROWS - FULL * P     # 24 leftover rows
NT = FULL + 1              # 18 scale columns (17 main t-slots + tail column)
G = 3                      # row-tiles per streaming super-tile
NBUF = 4                   # x-tile buffers
DROPOUT = 0.1
EPS = 1e-8
INVALID = -2.0


def _d2_threshold() -> float:
    # Largest f32 y with sqrt_f32(y) <= f32(0.1). Then for any f32 d2 >= 0:
    # (d2 > thr) == (sqrt_f32(d2) > f32(0.1)), so the kernel can compare
    # squared distances and still match the reference's sqrt-then-compare
    # bit for bit.
    c = np.float32(DROPOUT)
    lo = np.float32(0.005).view(np.uint32)
    hi = np.float32(0.02).view(np.uint32)
    while hi - lo > 1:
        mid = (lo + hi) // 2
        if np.sqrt(np.uint32(mid).view(np.float32)) <= c:
            lo = mid
        else:
            hi = mid
    return float(np.uint32(lo).view(np.float32))


# Row mapping: the 2176 "main" rows use row = 17*p + t (partition p, scale
# col t) so every DMA is per-partition contiguous; the last 24 rows are a
# separate tail unit at scale col 17 (partitions 0..23).
# streaming unit u -> (first t, number of t slots, first scale col)
def _units():
    units = []
    t = 0
    while t < FULL:
        g = min(G, FULL - t)
        units.append((t, g, t))
        t += g
    units.append((None, 1, FULL))                    # tail rows 2176..2199
    return units


def _build_nc() -> bass.Bass:
    f32 = mybir.dt.float32
    alu = mybir.AluOpType
    t2 = _d2_threshold()
    units = _units()
    NUNITS = len(units)

    nc = bass.Bass("TRN2")
    x_p = nc.declare_dram_parameter("x", [ROWS, T], f32, isOutput=False)
    pos_p = nc.declare_dram_parameter("positions", [ROWS, D], f32, isOutput=False)
    cen_p = nc.declare_dram_parameter("centers", [P, NCEN, D], f32, isOutput=False)
    out_p = nc.declare_dram_parameter("out", [ROWS, T], f32, isOutput=True)

    with ExitStack() as ctx:
        e = ctx.enter_context
        xbuf = [e(nc.sbuf_tensor(f"xbuf{i}", [P, G, T], f32)) for i in range(NBUF)]
        pos_all = e(nc.sbuf_tensor("pos_all", [P, NT, D], f32))
        cen_bc = e(nc.sbuf_tensor("cen_bc", [P, NCEN, D], f32))
        shape3 = [P, NT, NCEN]
        dx = e(nc.sbuf_tensor("dx", shape3, f32))
        dy = e(nc.sbuf_tensor("dy", shape3, f32))
        inv = e(nc.sbuf_tensor("inv", [P, NT], f32))
        eqy = e(nc.sbuf_tensor("eqy", [P, NT], f32))
        probs = e(nc.sbuf_tensor("probs", [P, NT], f32))
        denom = e(nc.sbuf_tensor("denom", [P, NT], f32))
        scale = e(nc.sbuf_tensor("scale", [P, NT], f32))

        lds = e(nc.semaphore("lds"))                       # phase-1 loads
        ld = [e(nc.semaphore(f"ld{i}")) for i in range(NBUF)]
        st = [e(nc.semaphore(f"st{i}")) for i in range(NBUF)]
        cv = e(nc.semaphore("cv"))                         # muls completed
        dv = e(nc.semaphore("dv"))                         # DVE op chain
        block = e(nc.Block())
        shared = {}

        def x_ap(dram, u):
            t0, g, _ = units[u]
            if t0 is None:  # tail
                return dram[FULL * P : ROWS, :]
            view = dram[0 : FULL * P, :].rearrange("(p t) m -> p t m", t=FULL)
            return view[:, t0 : t0 + g, :] if g > 1 else view[:, t0, :]

        def buf_ap(u):
            t0, g, _ = units[u]
            b = xbuf[u % NBUF]
            if t0 is None:
                return b[0:TAIL, 0, :]
            return b[:, 0:g, :] if g > 1 else b[:, 0, :]

        @block.sync
        def _(sync):
            # x loads, NBUF-deep. The three small phase-1 loads are issued
            # right AFTER the first big x load: x load 0 heads the DMA queue
            # at t~3us, the smalls slot in behind it (~0.35us of data), and
            # phase 1 still finishes with ~25us of slack before the first
            # store's queue position comes up.
            for u in range(NUNITS):
                if u >= NBUF:
                    sync.wait_ge(st[u % NBUF], 16 * (u // NBUF))
                sync.dma_start(out=buf_ap(u), in_=x_ap(x_p, u)).then_inc(
                    ld[u % NBUF], 16
                )
                if u == 0:
                    sync.dma_start(
                        out=pos_all[:, 0:FULL, :],
                        in_=pos_p[0 : FULL * P, :].rearrange(
                            "(p t) d -> p t d", t=FULL
                        ),
                    ).then_inc(lds, 16)
                    sync.wait_ge(dv, 1)  # tail slot memset done (DVE's 1st op)
                    sync.dma_start(
                        out=pos_all[0:TAIL, FULL, :],
                        in_=pos_p[FULL * P : ROWS, :],
                    ).then_inc(lds, 16)
                    sync.dma_start(out=cen_bc[:], in_=cen_p[:, :, :]).then_inc(
                        lds, 16
                    )

        @block.vector
        def _(vector):
            # dv counts completed DVE ops; each dependent op carries one
            # attached wait on its producer's count (waits gate dispatch, so
            # later instructions inherit earlier waits)
            n = 0

            def step(ins, wait=True):
                nonlocal n
                if wait:
                    ins._wait_ge(dv, n)
                ins.then_inc(dv, 1)
                n += 1

            # zero the tail pad (partitions 24..127 of scale col 17) that no
            # DMA writes; harmless values flow through phase 1 and are never
            # used by the tail multiply (it reads partitions 0..23 only)
            step(vector.memset(pos_all[:, FULL, :], 0.0), wait=False)
            vector.wait_ge(lds, 48)
            # invalid mask: both coords equal the sentinel
            step(vector.tensor_scalar(
                out=eqy[:, :], in0=pos_all[:, :, 1], scalar1=INVALID, scalar2=None,
                op0=alu.is_equal,
            ))
            step(vector.tensor_scalar(
                out=inv[:, :], in0=pos_all[:, :, 0], scalar1=INVALID, scalar2=None,
                op0=alu.is_equal,
            ), wait=False)
            step(vector.tensor_mul(inv[:, :], inv[:, :], eqy[:, :]))
            # squared distances to all centers
            cx_b = cen_bc[:, :, 0].unsqueeze(1).broadcast_to(shape3)
            cy_b = cen_bc[:, :, 1].unsqueeze(1).broadcast_to(shape3)
            px_b = pos_all[:, :, 0:1].broadcast_to(shape3)
            py_b = pos_all[:, :, 1:2].broadcast_to(shape3)
            step(vector.tensor_sub(dx[:], px_b, cx_b), wait=False)
            step(vector.tensor_sub(dy[:], py_b, cy_b), wait=False)
            step(vector.tensor_mul(dx[:], dx[:], dx[:]))
            step(vector.tensor_mul(dy[:], dy[:], dy[:]))
            step(vector.tensor_add(dx[:], dx[:], dy[:]))      # dx = dist^2
            # kept_all = (d2 > t2) | invalid  (masks are 1.0/0.0, max == or)
            step(vector.tensor_scalar(
                out=dy[:], in0=dx[:], scalar1=t2, scalar2=None, op0=alu.is_gt
            ))
            step(vector.tensor_max(
                dy[:], dy[:], inv[:, :].unsqueeze(2).broadcast_to(shape3)
            ))
            step(vector.tensor_reduce(
                probs[:, :], dy[:, :, 1:NCEN], axis=mybir.AxisListType.X, op=alu.add
            ))
            step(vector.tensor_scalar(
                out=denom[:, :], in0=probs[:, :], scalar1=1.0 / (NCEN - 1),
                scalar2=EPS, op0=alu.mult, op1=alu.add,
            ))
            step(vector.reciprocal(denom[:, :], denom[:, :]))
            # scale = kept0 * (1 - invalid) * 1/denom
            step(vector.tensor_scalar(
                out=inv[:, :], in0=inv[:, :], scalar1=-1.0, scalar2=1.0,
                op0=alu.mult, op1=alu.add,
            ))
            step(vector.tensor_mul(scale[:, :], dy[:, :, 0], inv[:, :]))
            step(vector.tensor_mul(scale[:, :], scale[:, :], denom[:, :]))
            n_phase1 = n
            shared["n_phase1"] = n_phase1
            # streaming multiplies: independent slices, each just needs its
            # load done (standalone wait) and scale done (first mul's attached
            # wait, inherited by the rest through dispatch order)
            first = True
            for u in range(NUNITS):
                t0, g, col0 = units[u]
                vector.wait_ge(ld[u % NBUF], 16 * (u // NBUF + 1))
                b = xbuf[u % NBUF]
                for gi in range(g):
                    if t0 is None:
                        ins = vector.tensor_scalar_mul(
                            b[0:TAIL, gi, :], b[0:TAIL, gi, :],
                            scale[0:TAIL, col0 + gi : col0 + gi + 1],
                        )
                    else:
                        ins = vector.tensor_scalar_mul(
                            b[:, gi, :], b[:, gi, :],
                            scale[:, col0 + gi : col0 + gi + 1],
                        )
                    if first:
                        ins._wait_ge(dv, n_phase1)
                        first = False
                    ins.then_inc(cv, 1)

        @block.scalar
        def _(scalar):
            cum = 0
            for u in range(NUNITS):
                cum += units[u][1]
                scalar.wait_ge(cv, cum)
                scalar.dma_start(out=x_ap(out_p, u), in_=buf_ap(u)).then_inc(
                    st[u % NBUF], 16
                )
            # make sure every store has landed before the program ends
            for i in range(NBUF):
                n_stores = len([u for u in range(NUNITS) if u % NBUF == i])
                scalar.wait_ge(st[i], 16 * n_stores)

    return nc


_nc_cache = None


def _get_nc() -> bass.Bass:
    global _nc_cache
    if _nc_cache is None:
        _nc_cache = _build_nc()
    return _nc_cache


def _in_maps(x, positions, centers):
    x = np.ascontiguousarray(x, dtype=np.float32)
    positions = np.ascontiguousarray(positions, dtype=np.float32)
    # centers are tiny and replicated; pre-broadcast across the 128 SBUF
    # partitions on the host so the device DMA is a plain [128, 202] load
    centers = np.ascontiguousarray(
        np.broadcast_to(np.asarray(centers, dtype=np.float32), (P, NCEN, D))
    )
    maps = []
    for i in range(N_CORES):
        maps.append(
            {
                "x": x[i * B_SH : (i + 1) * B_SH].reshape(ROWS, T),
                "positions": positions[i * B_SH : (i + 1) * B_SH].reshape(ROWS, D),
                "centers": centers,
            }
        )
    return maps


def run(x, positions, centers, **spmd_kwargs):
    """Run the SPMD kernel; returns (full_output, BassKernelResults)."""
    res = run_bass_kernel_spmd(
        _get_nc(), _in_maps(x, positions, centers), list(range(N_CORES)),
        **spmd_kwargs,
    )
    out = np.concatenate(
        [res.results[i]["out"].reshape(B_SH, C, T) for i in range(N_CORES)], axis=0
    )
    return out, res


def kernel(x, positions, centers):
    out, _ = run(x, positions, centers)
    return out


# revision 23
# speedup vs baseline: 1.0004x; 1.0004x over previous
"""ChannelDropout Bass kernel for 8 Trainium2 NeuronCores.

out[b,c,t] = x[b,c,t] * scale[b,c] with
  scale = valid * (dist0 > 0.1) / (mean_{n=1..100}((dist_n > 0.1)|~valid) + 1e-8)

Data-parallel over the batch dim: 8 batches per core; the per-core x shard
is a contiguous [2200, 3000] f32 block. Phase 1 computes the per-row scale
from positions/centers (tiny); phase 2 streams x through SBUF with a
per-partition scalar multiply (memory-bound, the roofline).

Written in raw Bass (not Tile): this toolchain's walrus build allows at
most ONE semaphore wait per instruction, which Tile's scheduler does not
respect. Also, consecutive instructions on the same engine have no RAW
interlock, so dependent compute ops are chained through a progress
semaphore. Pipeline: sync engine issues x loads (HWDGE/SP ring), vector
engine does all compute, scalar engine issues stores (HWDGE/ACT ring),
NBUF-deep buffered SBUF x tiles with per-slot DMA sems.
"""

from contextlib import ExitStack

import numpy as np

import concourse.bass as bass
from concourse import mybir
from concourse.bass_utils import run_bass_kernel_spmd

B, C, T, D = 64, 275, 3000, 2
NCEN = 101                 # centers rows (0 = ban center, 1..100 Monte-Carlo)
N_CORES = 8
B_SH = B // N_CORES        # 8 batches per core
ROWS = B_SH * C            # 2200 rows per core
P = 128
FULL = ROWS // P           # 17 full 128-row tiles
TAIL = ROWS - FULL * P     # 24 leftover rows
NT = FULL + 1              # 18 scale columns (17 main t-slots + tail column)
G = 3                      # row-tiles per streaming super-tile
NBUF = 4                   # x-tile buffers
DROPOUT = 0.1
EPS = 1e-8
INVALID = -2.0


def _d2_threshold() -> float:
    # Largest f32 y with sqrt_f32(y) <= f32(0.1). Then for any f32 d2 >= 0:
    # (d2 > thr) == (sqrt_f32(d2) > f32(0.1)), so the kernel can compare
    # squared distances and still match the reference's sqrt-then-compare
    # bit for bit.
    c = np.float32(DROPOUT)
    lo = np.float32(0.005).view(np.uint32)
    hi = np.float32(0.02).view(np.uint32)
    while hi - lo > 1:
        mid = (lo + hi) // 2
        if np.sqrt(np.uint32(mid).view(np.float32)) <= c:
            lo = mid
        else:
            hi = mid
    return float(np.uint32(lo).view(np.float32))


# Row mapping: the 2176 "main" rows use row = 17*p + t (partition p, scale
# col t) so every DMA is per-partition contiguous; the last 24 rows are a
# separate tail unit at scale col 17 (partitions 0..23).
# streaming unit u -> (first t, number of t slots, first scale col)
def _units():
    units = []
    t = 0
    while t < FULL:
        g = min(G, FULL - t)
        units.append((t, g, t))
        t += g
    units.append((None, 1, FULL))                    # tail rows 2176..2199
    return units


def _build_nc() -> bass.Bass:
    f32 = mybir.dt.float32
    alu = mybir.AluOpType
    t2 = _d2_threshold()
    units = _units()
    NUNITS = len(units)

    nc = bass.Bass("TRN2", monotonic_sem_count=0)
    x_p = nc.declare_dram_parameter("x", [ROWS, T], f32, isOutput=False)
    pos_p = nc.declare_dram_parameter("positions", [ROWS, D], f32, isOutput=False)
    cen_p = nc.declare_dram_parameter("centers", [P, NCEN, D], f32, isOutput=False)
    out_p = nc.declare_dram_parameter("out", [ROWS, T], f32, isOutput=True)

    with ExitStack() as ctx:
        e = ctx.enter_context
        xbuf = [e(nc.sbuf_tensor(f"xbuf{i}", [P, G, T], f32)) for i in range(NBUF)]
        pos_all = e(nc.sbuf_tensor("pos_all", [P, NT, D], f32))
        cen_bc = e(nc.sbuf_tensor("cen_bc", [P, NCEN, D], f32))
        shape3 = [P, NT, NCEN]
        dx = e(nc.sbuf_tensor("dx", shape3, f32))
        dy = e(nc.sbuf_tensor("dy", shape3, f32))
        inv = e(nc.sbuf_tensor("inv", [P, NT], f32))
        eqy = e(nc.sbuf_tensor("eqy", [P, NT], f32))
        probs = e(nc.sbuf_tensor("probs", [P, NT], f32))
        denom = e(nc.sbuf_tensor("denom", [P, NT], f32))
        scale = e(nc.sbuf_tensor("scale", [P, NT], f32))

        lds = e(nc.semaphore("lds"))                       # phase-1 loads
        ld = [e(nc.semaphore(f"ld{i}")) for i in range(NBUF)]
        st = [e(nc.semaphore(f"st{i}")) for i in range(NBUF)]
        cv = e(nc.semaphore("cv"))                         # muls completed
        dv = e(nc.semaphore("dv"))                         # DVE op chain
        block = e(nc.Block())
        shared = {}

        def x_ap(dram, u):
            t0, g, _ = units[u]
            if t0 is None:  # tail
                return dram[FULL * P : ROWS, :]
            view = dram[0 : FULL * P, :].rearrange("(p t) m -> p t m", t=FULL)
            return view[:, t0 : t0 + g, :] if g > 1 else view[:, t0, :]

        def buf_ap(u):
            t0, g, _ = units[u]
            b = xbuf[u % NBUF]
            if t0 is None:
                return b[0:TAIL, 0, :]
            return b[:, 0:g, :] if g > 1 else b[:, 0, :]

        @block.sync
        def _(sync):
            # x loads, NBUF-deep. The three small phase-1 loads are issued
            # right AFTER the first big x load: x load 0 heads the DMA queue
            # at t~3us, the smalls slot in behind it (~0.35us of data), and
            # phase 1 still finishes with ~25us of slack before the first
            # store's queue position comes up.
            for u in range(NUNITS):
                if u >= NBUF:
                    sync.wait_ge(st[u % NBUF], 16 * (u // NBUF))
                sync.dma_start(out=buf_ap(u), in_=x_ap(x_p, u)).then_inc(
                    ld[u % NBUF], 16
                )
                if u == 0:
                    sync.dma_start(
                        out=pos_all[:, 0:FULL, :],
                        in_=pos_p[0 : FULL * P, :].rearrange(
                            "(p t) d -> p t d", t=FULL
                        ),
                    ).then_inc(lds, 16)
                    sync.wait_ge(dv, 1)  # tail slot memset done (DVE's 1st op)
                    sync.dma_start(
                        out=pos_all[0:TAIL, FULL, :],
                        in_=pos_p[FULL * P : ROWS, :],
                    ).then_inc(lds, 16)
                    sync.dma_start(out=cen_bc[:], in_=cen_p[:, :, :]).then_inc(
                        lds, 16
                    )

        @block.vector
        def _(vector):
            # dv counts completed DVE ops; each dependent op carries one
            # attached wait on its producer's count (waits gate dispatch, so
            # later instructions inherit earlier waits)
            n = 0

            def step(ins, wait=True):
                nonlocal n
                if wait:
                    ins._wait_ge(dv, n)
                ins.then_inc(dv, 1)
                n += 1

            # zero the tail pad (partitions 24..127 of scale col 17) that no
            # DMA writes; harmless values flow through phase 1 and are never
            # used by the tail multiply (it reads partitions 0..23 only)
            step(vector.memset(pos_all[:, FULL, :], 0.0), wait=False)
            vector.wait_ge(lds, 48)
            # invalid mask: both coords equal the sentinel
            step(vector.tensor_scalar(
                out=eqy[:, :], in0=pos_all[:, :, 1], scalar1=INVALID, scalar2=None,
                op0=alu.is_equal,
            ))
            step(vector.tensor_scalar(
                out=inv[:, :], in0=pos_all[:, :, 0], scalar1=INVALID, scalar2=None,
                op0=alu.is_equal,
            ), wait=False)
            step(vector.tensor_mul(inv[:, :], inv[:, :], eqy[:, :]))
            # squared distances to all centers
            cx_b = cen_bc[:, :, 0].unsqueeze(1).broadcast_to(shape3)
            cy_b = cen_bc[:, :, 1].unsqueeze(1).broadcast_to(shape3)
            px_b = pos_all[:, :, 0:1].broadcast_to(shape3)
            py_b = pos_all[:, :, 1:2].broadcast_to(shape3)
            step(vector.tensor_sub(dx[:], px_b, cx_b), wait=False)
            step(vector.tensor_sub(dy[:], py_b, cy_b), wait=False)
            step(vector.tensor_mul(dx[:], dx[:], dx[:]))
            step(vector.tensor_mul(dy[:], dy[:], dy[:]))
            step(vector.tensor_add(dx[:], dx[:], dy[:]))      # dx = dist^2
            # kept_all = (d2 > t2) | invalid  (masks are 1.0/0.0, max == or)
            step(vector.tensor_scalar(
                out=dy[:], in0=dx[:], scalar1=t2, scalar2=None, op0=alu.is_gt
            ))
            step(vector.tensor_max(
                dy[:], dy[:], inv[:, :].unsqueeze(2).broadcast_to(shape3)
            ))
            step(vector.tensor_reduce(
                probs[:, :], dy[:, :, 1:NCEN], axis=mybir.AxisListType.X, op=alu.add
            ))
            step(vector.tensor_scalar(
                out=denom[:, :], in0=probs[:, :], scalar1=1.0 / (NCEN - 1),
                scalar2=EPS, op0=alu.mult, op1=alu.add,
            ))
            step(vector.reciprocal(denom[:, :], denom[:, :]))
            # scale = kept0 * (1 - invalid) * 1/denom
            step(vector.tensor_scalar(
                out=inv[:, :], in0=inv[:, :], scalar1=-1.0, scalar2=1.0,
                op0=alu.mult, op1=alu.add,
            ))
            step(vector.tensor_mul(scale[:, :], dy[:, :, 0], inv[:, :]))
            step(vector.tensor_mul(scale[:, :], scale[:, :], denom[:, :]))
            n_phase1 = n
            shared["n_phase1"] = n_phase1
            # streaming multiplies: independent slices, each just needs its
            # load done (standalone wait) and scale done (first mul's attached
            # wait, inherited by the rest through dispatch order)
            first = True
            for u in range(NUNITS):
                t0, g, col0 = units[u]
                vector.wait_ge(ld[u % NBUF], 16 * (u // NBUF + 1))
                b = xbuf[u % NBUF]
                for gi in range(g):
                    if t0 is None:
                        ins = vector.tensor_scalar_mul(
                            b[0:TAIL, gi, :], b[0:TAIL, gi, :],
                            scale[0:TAIL, col0 + gi : col0 + gi + 1],
                        )
                    else:
                        ins = vector.tensor_scalar_mul(
                            b[:, gi, :], b[:, gi, :],
                            scale[:, col0 + gi : col0 + gi + 1],
                        )
                    if first:
                        ins._wait_ge(dv, n_phase1)
                        first = False
                    ins.then_inc(cv, 1)

        @block.scalar
        def _(scalar):
            cum = 0
            for u in range(NUNITS):
                cum += units[u][1]
                scalar.wait_ge(cv, cum)
                scalar.dma_start(out=x_ap(out_p, u), in_=buf_ap(u)).then_inc(
                    st[u % NBUF], 16
                )
            # make sure every store has landed before the program ends
            for i in range(NBUF):
                n_stores = len([u for u in range(NUNITS) if u % NBUF == i])
                scalar.wait_ge(st[i], 16 * n_stores)

    return nc


_nc_cache = None


def _get_nc() -> bass.Bass:
    global _nc_cache
    if _nc_cache is None:
        _nc_cache = _build_nc()
    return _nc_cache


def _in_maps(x, positions, centers):
    x = np.ascontiguousarray(x, dtype=np.float32)
    positions = np.ascontiguousarray(positions, dtype=np.float32)
    # centers are tiny and replicated; pre-broadcast across the 128 SBUF
    # partitions on the host so the device DMA is a plain [128, 202] load
    centers = np.ascontiguousarray(
        np.broadcast_to(np.asarray(centers, dtype=np.float32), (P, NCEN, D))
    )
    maps = []
    for i in range(N_CORES):
        maps.append(
            {
                "x": x[i * B_SH : (i + 1) * B_SH].reshape(ROWS, T),
                "positions": positions[i * B_SH : (i + 1) * B_SH].reshape(ROWS, D),
                "centers": centers,
            }
        )
    return maps


def run(x, positions, centers, **spmd_kwargs):
    """Run the SPMD kernel; returns (full_output, BassKernelResults)."""
    res = run_bass_kernel_spmd(
        _get_nc(), _in_maps(x, positions, centers), list(range(N_CORES)),
        **spmd_kwargs,
    )
    out = np.concatenate(
        [res.results[i]["out"].reshape(B_SH, C, T) for i in range(N_CORES)], axis=0
    )
    return out, res


def kernel(x, positions, centers):
    out, _ = run(x, positions, centers)
    return out
